# revision 27
# baseline (speedup 1.0000x reference)
"""Trainium2 Bass kernel for nn_AttentionModel (S=2048, B=32, H=1024).

Math: reference computes
    energy[b,s] = (enc[s,b,:] @ We.T + (h @ Wh.T + bias)) @ v  ; out = softmax_s(energy)
Since softmax is shift-invariant and the (h @ Wh.T + bias) @ v term is constant
over s, the output reduces exactly to
    out[b, 0, s] = softmax_s( enc[s,b,:] . u ),   u = v[0] @ We   (We = attn_W[:, H:])
So the kernel is a memory-bound [S*B, H] x [H] matvec + row softmax.

Sharding: data-parallel over batch B across 8 cores (4 batches/core). All
device data is fp16: it halves HBM traffic (the roofline) and the induced
energy noise (~8e-3 abs) is far inside the 2e-2 gate.

Engine split per core: the PE computes batches 1-3 (enc laid out [H, S]: h on
SBUF partitions, contraction in chunks of 128, lhsT = u chunk [128,1], rhs =
enc tile [128,512], PSUM-accumulated). Batch 0 goes to the otherwise-idle
Vector engine via fused tensor_tensor_reduce (enc laid out [S, H]: s on
partitions, multiply by a partition-replicated u and reduce over h). The DVE
offload cuts PE work 25% so runs where DVFS power-throttling halves the PE
clock (which otherwise makes the PE the critical path) stay DMA-bound.

Softmax: the device returns num[b,s] = exp(energy - C) for a constant C ~
3.6*||u|| (shift-invariance; C keeps exp in fp32 range since energy ~
N(0, ||u||^2)). The host divides by the f64 row sum. No reduce_max, no
on-device sums -> the post-stream tail is just matmul -> exp -> 2KB DMA out.

DMA: enc prefetch on the Sync engine's HWDGE ring (fine 512KB chunks so PE
dependencies are fine-grained; deep tile pool so trigger issue is never gated
on PE consumption); outputs on the Activation engine's own HWDGE ring so out
triggers never block enc prefetch triggers. The unused SWDGE ring declaration
is dropped.
"""

import numpy as np

import concourse.bass as bass
import concourse.tile as tile
from concourse import bacc, mybir
from concourse.bass_utils import run_bass_kernel_spmd

S, B, H = 2048, 32, 1024
NCORES = 8
BL = B // NCORES  # batches per core
MM_N = 512        # matmul moving free dim (fp32 max, 1 PSUM bank)


def build_nc(bl=BL, h=H, s=S, enc_bufs=20, jpd=1, mm_dtype="float16",
             taper=True, strip_queues=True, dve_batches=1):
    """Build the per-core Bass program (SPMD: same program, different data)."""
    nc = bacc.Bacc()
    f32 = mybir.dt.float32
    jc = h // 128      # h chunks (PE contraction tiles)
    ns = s // MM_N     # matmul slices per output row
    st = s // 128      # s tiles (DVE path)
    jpd = min(jpd, jc)
    nb = bl - dve_batches  # PE batches
    nd = jc // jpd
    plan = [[jpd] * nd for _ in range(nb)]
    split_last = taper and jpd == 1

    mm_dt = getattr(mybir.dt, mm_dtype)
    enc_d = nc.declare_dram_parameter("enc", [nb, h, s], mm_dt, isOutput=False)
    enc0_d = nc.declare_dram_parameter("enc0", [s, h], mm_dt, isOutput=False)
    u_d = nc.declare_dram_parameter("u", [128, jc], mm_dt, isOutput=False)
    uv_d = nc.declare_dram_parameter("uv", [128, h], mm_dt, isOutput=False)
    cb_d = nc.declare_dram_parameter("cb", [128, 1], f32, isOutput=False)
    out_d = nc.declare_dram_parameter("out", [nb, s], f32, isOutput=True)
    out0_d = nc.declare_dram_parameter("out0", [128, st], f32, isOutput=True)

    n_pe_chunks = sum(len(p) for p in plan)

    with tile.TileContext(nc) as tc:
        with (
            tc.tile_pool(name="up", bufs=1) as up,
            tc.tile_pool(name="encp", bufs=enc_bufs) as encp,
            tc.tile_pool(name="dvp", bufs=4) as dvp,
            tc.tile_pool(name="smp", bufs=bl) as smp,
            tc.tile_pool(name="psp", bufs=2, space="PSUM") as psp,
        ):
            # First enc chunk before anything else so the DMA pipeline (and
            # the PE) start immediately; the small param loads follow.
            t0 = encp.tile([128, plan[0][0], s], mm_dt, name="t",
                           padded_shape=[128, jpd, s])
            nc.sync.dma_start(
                t0[:],
                enc_d[0, 0:plan[0][0] * 128, :].rearrange("(j p) s -> p j s", p=128),
            )
            u_sb = up.tile([128, jc], mm_dt)
            nc.sync.dma_start(u_sb[:], u_d[:])
            uv_sb = up.tile([128, h], mm_dt)
            nc.sync.dma_start(uv_sb[:], uv_d[:])
            cb_sb = up.tile([128, 1], f32)
            nc.sync.dma_start(cb_sb[:], cb_d[:])

            eb = up.tile([128, st], f32)      # DVE energies, s on partitions
            tmp = up.tile([128, h], mm_dt)    # DVE elementwise product (junk)
            p0 = up.tile([128, st], f32)

            pe_issued = 0
            dv_issued = [0]

            def issue_dve_share():
                # Front-load the DVE tile stream (2 tiles per PE chunk, all
                # issued by PE chunk 8) so the DVE finishes early; the
                # interleave only exists so the PE's first chunks aren't
                # queued behind 4MB of batch-0 data.
                target = min(st, pe_issued * 2)
                while dv_issued[0] < target:
                    k = dv_issued[0]
                    dt_ = dvp.tile([128, h], mm_dt, name="d0")
                    nc.sync.dma_start(dt_[:], enc0_d[k * 128:(k + 1) * 128, :])
                    # (tensor_tensor_reduce would fuse these, but its
                    # lowering faults the device on this stack.)
                    nc.vector.tensor_tensor(tmp[:], dt_[:], uv_sb[:],
                                            op=mybir.AluOpType.mult)
                    nc.vector.tensor_reduce(eb[:, k:k + 1], tmp[:],
                                            axis=mybir.AxisListType.X,
                                            op=mybir.AluOpType.add)
                    dv_issued[0] += 1

            for b in range(nb):
                # Accumulate this batch's energy row in PSUM [1, s] (4 banks,
                # partition 0); 8 matmuls per 512-wide slice.
                e_ps = psp.tile([1, s], f32)
                p_exp = smp.tile([1, s], f32)
                last = b == nb - 1 and split_last
                if last:
                    # Batch 0's DVE results are complete well before the PE
                    # tail; emitting its (cheap, [128, st]) exp + out-DMA
                    # here keeps them off the end of the Scalar stream.
                    nc.scalar.activation(
                        p0[:], eb[:], mybir.ActivationFunctionType.Exp,
                        bias=cb_sb[:],
                    )
                    nc.scalar.dma_start(out0_d[:], p0[:])
                j = 0
                for d, cw in enumerate(plan[b]):
                    # The last batch's final h-chunk is streamed as per-slice
                    # sub-DMAs (the very last slice split once more) so only
                    # ONE small matmul + exp + out-DMA sit after the final
                    # bytes of the stream.
                    taper_d = last and d == len(plan[b]) - 1
                    if taper_d:
                        pieces = [(k * MM_N, MM_N) for k in range(ns - 1)]
                        pieces += [((ns - 1) * MM_N, MM_N // 2),
                                   ((ns - 1) * MM_N + MM_N // 2, MM_N // 2)]
                    else:
                        pieces = [(0, s)]
                    for off, w in pieces:
                        if b == 0 and d == 0:
                            t = t0
                            coff = 0
                        else:
                            t = encp.tile([128, cw, w], mm_dt, name="t",
                                          padded_shape=[128, jpd, s])
                            src = enc_d[b, j * 128:(j + cw) * 128,
                                        off:off + w]
                            nc.sync.dma_start(
                                t[:], src.rearrange("(j p) s -> p j s", p=128)
                            )
                            coff = -off
                        pe_issued += 1
                        issue_dve_share()
                        for jl in range(cw):
                            if taper_d:
                                subs = [(off, w)]
                            else:
                                subs = [(k * MM_N, MM_N) for k in range(ns)]
                            for soff, sw in subs:
                                nc.tensor.matmul(
                                    e_ps[:, soff:soff + sw],
                                    u_sb[:, j + jl:j + jl + 1],
                                    t[:, jl, soff + coff:soff + coff + sw],
                                    start=(j + jl == 0),
                                    stop=(j + jl == jc - 1),
                                )
                                if j + jl == jc - 1:
                                    # This region's accumulation is complete:
                                    # exp(e - C), overlapping remaining
                                    # matmuls/DMAs. Output DMAs ride the
                                    # Activation engine's own HWDGE ring so
                                    # they never block the Sync engine's enc
                                    # prefetch triggers.
                                    nc.scalar.activation(
                                        p_exp[:, soff:soff + sw],
                                        e_ps[:, soff:soff + sw],
                                        mybir.ActivationFunctionType.Exp,
                                        bias=cb_sb[0:1, :],
                                    )
                                    if last:
                                        nc.scalar.dma_start(
                                            out_d[b:b + 1, soff:soff + sw],
                                            p_exp[:, soff:soff + sw],
                                        )
                    j += cw
                if not last:
                    nc.scalar.dma_start(out_d[b:b + 1, :], p_exp[:])
    if strip_queues:
        # The SWDGE ring (qPoolDynamic) is unused - drop its declaration.
        nc.m.queues = [q for q in nc.m.queues if q.name != "qPoolDynamic"]
    nc.compile()
    return nc


def _prep_inputs(encoder_outputs, attn_W, v, np_dtype=np.float16):
    encoder_outputs = np.asarray(encoder_outputs, dtype=np.float32)
    attn_W = np.asarray(attn_W, dtype=np.float32)
    v = np.asarray(v, dtype=np.float32)
    h = attn_W.shape[0]
    # u = v[0] @ We in float64 (host-side, tiny)
    u64 = v[0].astype(np.float64) @ attn_W[:, h:].astype(np.float64)
    u = u64.astype(np_dtype)
    # energy[b,s] ~ N(0, ||u||^2); C ~ expected row max keeps exp() in range.
    bias_c = 3.6 * float(np.linalg.norm(u64))
    u128 = np.ascontiguousarray(u.reshape(h // 128, 128).T)  # [128, jc]
    uv = np.ascontiguousarray(np.broadcast_to(u, (128, h)))
    cb = np.full((128, 1), -bias_c, dtype=np.float32)
    in_maps = []
    for c in range(NCORES):
        sl = encoder_outputs[:, c * BL:(c + 1) * BL, :]
        enc0 = np.ascontiguousarray(sl[:, 0, :].astype(np_dtype))       # [S,H]
        rest = np.ascontiguousarray(
            sl[:, 1:, :].transpose(1, 2, 0).astype(np_dtype))           # [nb,H,S]
        in_maps.append({"enc": rest, "enc0": enc0, "u": u128, "uv": uv,
                        "cb": cb})
    return in_maps, bias_c


def run(encoder_outputs, rnn_hidden, attn_W, attn_b, v, trace=False,
        mm_dtype="float16", **bass_kwargs):
    np_dtype = {"float16": np.float16, "float32r": np.float32,
                "float32": np.float32}[mm_dtype]
    in_maps, bias_c = _prep_inputs(encoder_outputs, attn_W, v, np_dtype=np_dtype)
    nc = build_nc(mm_dtype=mm_dtype)
    res = run_bass_kernel_spmd(
        nc, in_maps, list(range(NCORES)), trace=trace, **bass_kwargs
    )
    rows = []
    for r in res.results:
        b0 = r["out0"].T.reshape(1, -1)   # [128, st] -> s = k*128 + p
        rows.append(np.concatenate([b0, r["out"]], axis=0))  # [BL, S]
    num = np.concatenate(rows, axis=0)                       # [B, S]
    tot = num.astype(np.float64).sum(axis=1)                 # [B]
    out = num / tot[:, None]
    return out[:, None, :].astype(np.float32), res


def kernel(encoder_outputs, rnn_hidden, attn_W, attn_b, v):
    out, _ = run(encoder_outputs, rnn_hidden, attn_W, attn_b, v)
    return out


# revision 28
# speedup vs baseline: 1.0866x; 1.0866x over previous
"""Trainium2 Bass kernel for nn_AttentionModel (S=2048, B=32, H=1024).

Math: reference computes
    energy[b,s] = (enc[s,b,:] @ We.T + (h @ Wh.T + bias)) @ v  ; out = softmax_s(energy)
Since softmax is shift-invariant and the (h @ Wh.T + bias) @ v term is constant
over s, the output reduces exactly to
    out[b, 0, s] = softmax_s( enc[s,b,:] . u ),   u = v[0] @ We   (We = attn_W[:, H:])
So the kernel is a memory-bound [S*B, H] x [H] matvec + row softmax.

Sharding: data-parallel over batch B across 8 cores (4 batches/core).
Device layout per core: enc [BL, H, S] in fp16 (h on SBUF partitions, s on free
dim) - fp16 halves HBM traffic (the roofline) and the PE matmul runs at the
same 1 cycle/row as fp32r; the induced energy noise (~8e-3 abs) is far inside
the 2e-2 gate. PE contracts h in chunks of 128 (lhsT = u chunk [128,1], rhs =
enc tile [128,512], PSUM-accumulated).

Softmax: the device returns num[b,s] = exp(energy - C) for a constant C ~
3.6*||u|| (softmax shift-invariance; C keeps exp in fp32 range since energy
~ N(0, ||u||^2)). The host divides by the f64 row sum. No reduce_max, no
on-device sums -> the Vector engine does nothing and the post-stream tail is
just matmul -> exp -> small DMA out.

DMA: enc prefetch on the Sync engine's HWDGE ring in fine 512KB single-h-chunk
DMAs (fine-grained PE dependencies; a deep 20-buffer tile pool so trigger
issue is never gated on PE consumption). Output DMAs ride the Activation
engine's own HWDGE ring - the trigger sits right after the exp in the
Activation stream and never blocks the Sync engine's enc prefetch triggers.
The unused SWDGE ring declaration is dropped. The last batch's final h-chunk
streams as per-slice sub-DMAs (the very last slice split once more) so only
one small matmul + exp + out-DMA sit after the final bytes of the stream.
"""

import numpy as np

import concourse.bass as bass
import concourse.tile as tile
from concourse import bacc, mybir
from concourse.bass_utils import run_bass_kernel_spmd

S, B, H = 2048, 32, 1024
NCORES = 8
BL = B // NCORES  # batches per core
MM_N = 512        # matmul moving free dim (fp32 max, 1 PSUM bank)


def build_nc(bl=BL, h=H, s=S, enc_bufs=20, jpd=1, mm_dtype="float16",
             taper=True, strip_queues=True):
    """Build the per-core Bass program (SPMD: same program, different data)."""
    nc = bacc.Bacc()
    f32 = mybir.dt.float32
    jc = h // 128      # h chunks (contraction tiles)
    ns = s // MM_N     # matmul slices per output row
    jpd = min(jpd, jc) # h-chunks per DMA
    nd = jc // jpd     # DMAs per batch
    plan = [[jpd] * nd for _ in range(bl)]
    split_last = taper and jpd == 1

    mm_dt = getattr(mybir.dt, mm_dtype)
    enc_d = nc.declare_dram_parameter("enc", [bl, h, s], mm_dt, isOutput=False)
    u_d = nc.declare_dram_parameter("u", [128, jc], mm_dt, isOutput=False)
    cb_d = nc.declare_dram_parameter("cb", [1, 1], f32, isOutput=False)
    out_d = nc.declare_dram_parameter("out", [bl, s], f32, isOutput=True)

    with tile.TileContext(nc) as tc:
        with (
            tc.tile_pool(name="up", bufs=1) as up,
            tc.tile_pool(name="encp", bufs=enc_bufs) as encp,
            tc.tile_pool(name="smp", bufs=bl) as smp,
            tc.tile_pool(name="psp", bufs=2, space="PSUM") as psp,
        ):
            # Issue the first enc load before anything else so the DMA
            # pipeline starts immediately; the tiny u/cb loads follow it.
            t0 = encp.tile([128, plan[0][0], s], mm_dt, name="t",
                           padded_shape=[128, jpd, s])
            nc.sync.dma_start(
                t0[:],
                enc_d[0, 0:plan[0][0] * 128, :].rearrange("(j p) s -> p j s", p=128),
            )
            u_sb = up.tile([128, jc], mm_dt)
            nc.sync.dma_start(u_sb[:], u_d[:])
            cb_sb = up.tile([1, 1], f32)
            nc.sync.dma_start(cb_sb[:], cb_d[:])

            for b in range(bl):
                # Accumulate this batch's energy row in PSUM [1, s] (4 banks,
                # partition 0); 8 matmuls per 512-wide slice.
                e_ps = psp.tile([1, s], f32)
                p_exp = smp.tile([1, s], f32)
                last = b == bl - 1 and split_last
                j = 0
                for d, cw in enumerate(plan[b]):
                    taper_d = last and d == len(plan[b]) - 1
                    if taper_d:
                        pieces = [(k * MM_N, MM_N) for k in range(ns - 1)]
                        pieces += [((ns - 1) * MM_N, MM_N // 2),
                                   ((ns - 1) * MM_N + MM_N // 2, MM_N // 2)]
                    else:
                        pieces = [(0, s)]
                    for off, w in pieces:
                        if b == 0 and d == 0:
                            t = t0
                            coff = 0
                        else:
                            t = encp.tile([128, cw, w], mm_dt, name="t",
                                          padded_shape=[128, jpd, s])
                            src = enc_d[b, j * 128:(j + cw) * 128,
                                        off:off + w]
                            nc.sync.dma_start(
                                t[:], src.rearrange("(j p) s -> p j s", p=128)
                            )
                            coff = -off
                        for jl in range(cw):
                            if taper_d:
                                subs = [(off, w)]
                            else:
                                subs = [(k * MM_N, MM_N) for k in range(ns)]
                            for soff, sw in subs:
                                nc.tensor.matmul(
                                    e_ps[:, soff:soff + sw],
                                    u_sb[:, j + jl:j + jl + 1],
                                    t[:, jl, soff + coff:soff + coff + sw],
                                    start=(j + jl == 0),
                                    stop=(j + jl == jc - 1),
                                )
                                if j + jl == jc - 1:
                                    # This region's accumulation is complete:
                                    # exp(e - C), overlapping remaining
                                    # matmuls/DMAs.
                                    nc.scalar.activation(
                                        p_exp[:, soff:soff + sw],
                                        e_ps[:, soff:soff + sw],
                                        mybir.ActivationFunctionType.Exp,
                                        bias=cb_sb[:],
                                    )
                                    if last:
                                        nc.scalar.dma_start(
                                            out_d[b:b + 1, soff:soff + sw],
                                            p_exp[:, soff:soff + sw],
                                        )
                    j += cw
                if not last:
                    nc.scalar.dma_start(out_d[b:b + 1, :], p_exp[:])
    if strip_queues:
        # The SWDGE ring (qPoolDynamic) is unused - drop its declaration.
        nc.m.queues = [q for q in nc.m.queues if q.name != "qPoolDynamic"]
    nc.compile()
    return nc


def _prep_inputs(encoder_outputs, attn_W, v, np_dtype=np.float16):
    encoder_outputs = np.asarray(encoder_outputs, dtype=np.float32)
    attn_W = np.asarray(attn_W, dtype=np.float32)
    v = np.asarray(v, dtype=np.float32)
    h = attn_W.shape[0]
    # u = v[0] @ We in float64 (host-side, tiny)
    u64 = v[0].astype(np.float64) @ attn_W[:, h:].astype(np.float64)
    u = u64.astype(np_dtype)
    # energy[b,s] ~ N(0, ||u||^2); C ~ expected row max keeps exp() in range.
    bias_c = 3.6 * float(np.linalg.norm(u64))
    u128 = np.ascontiguousarray(u.reshape(h // 128, 128).T)  # [128, jc]
    cb = np.array([[-bias_c]], dtype=np.float32)
    in_maps = []
    for c in range(NCORES):
        sl = encoder_outputs[:, c * BL:(c + 1) * BL, :]
        enc_c = np.ascontiguousarray(sl.transpose(1, 2, 0).astype(np_dtype))  # [BL,H,S]
        in_maps.append({"enc": enc_c, "u": u128, "cb": cb})
    return in_maps, bias_c


def run(encoder_outputs, rnn_hidden, attn_W, attn_b, v, trace=False,
        mm_dtype="float16", **bass_kwargs):
    np_dtype = {"float16": np.float16, "float32r": np.float32,
                "float32": np.float32}[mm_dtype]
    in_maps, bias_c = _prep_inputs(encoder_outputs, attn_W, v, np_dtype=np_dtype)
    nc = build_nc(mm_dtype=mm_dtype)
    res = run_bass_kernel_spmd(
        nc, in_maps, list(range(NCORES)), trace=trace, **bass_kwargs
    )
    num = np.concatenate([r["out"] for r in res.results], axis=0)  # [B, S]
    tot = num.astype(np.float64).sum(axis=1)                       # [B]
    out = num / tot[:, None]
    return out[:, None, :].astype(np.float32), res


def kernel(encoder_outputs, rnn_hidden, attn_W, attn_b, v):
    out, _ = run(encoder_outputs, rnn_hidden, attn_W, attn_b, v)
    return out


# revision 29
# speedup vs baseline: 1.1161x; 1.0272x over previous
"""Trainium2 Bass kernel for nn_AttentionModel (S=2048, B=32, H=1024).

Math: reference computes
    energy[b,s] = (enc[s,b,:] @ We.T + (h @ Wh.T + bias)) @ v  ; out = softmax_s(energy)
Since softmax is shift-invariant and the (h @ Wh.T + bias) @ v term is constant
over s, the output reduces exactly to
    out[b, 0, s] = softmax_s( enc[s,b,:] . u ),   u = v[0] @ We   (We = attn_W[:, H:])
So the kernel is a memory-bound [S*B, H] x [H] matvec + row softmax.

Sharding: data-parallel over batch B across 8 cores (4 batches/core).
Device layout per core: enc [BL, H, S] in fp16 (h on SBUF partitions, s on free
dim) - fp16 halves HBM traffic (the roofline) and the PE matmul runs at the
same 1 cycle/row as fp32r; the induced energy noise (~8e-3 abs) is far inside
the 2e-2 gate. PE contracts h in chunks of 128 (lhsT = u chunk [128,1], rhs =
enc tile [128,512], PSUM-accumulated).

Softmax: the device returns num[b,s] = exp(energy - C) for a constant C ~
3.6*||u|| (softmax shift-invariance; C keeps exp in fp32 range since energy
~ N(0, ||u||^2)). The host divides by the f64 row sum. No reduce_max, no
on-device sums -> the Vector engine does nothing and the post-stream tail is
just matmul -> exp -> small DMA out.

DMA: enc prefetch on the Sync engine's HWDGE ring in fine 512KB single-h-chunk
DMAs (fine-grained PE dependencies; a deep 20-buffer tile pool so trigger
issue is never gated on PE consumption). Output DMAs ride the Activation
engine's own HWDGE ring - the trigger sits right after the exp in the
Activation stream and never blocks the Sync engine's enc prefetch triggers.
The unused SWDGE ring declaration is dropped. The last batch's final h-chunk
streams as per-slice sub-DMAs (the very last slice split once more) so only
one small matmul + exp + out-DMA sit after the final bytes of the stream.
"""

import numpy as np

import concourse.bass as bass
import concourse.tile as tile
from concourse import bacc, mybir
from concourse.bass_utils import run_bass_kernel_spmd

S, B, H = 2048, 32, 1024
NCORES = 8
BL = B // NCORES  # batches per core
MM_N = 512        # matmul moving free dim (fp32 max, 1 PSUM bank)


def build_nc(bl=BL, h=H, s=S, enc_bufs=8, jpd=4, mm_dtype="float16",
             taper=True, strip_queues=True):
    """Build the per-core Bass program (SPMD: same program, different data)."""
    nc = bacc.Bacc()
    f32 = mybir.dt.float32
    jc = h // 128      # h chunks (contraction tiles)
    ns = s // MM_N     # matmul slices per output row
    jpd = min(jpd, jc) # h-chunks per DMA
    nd = jc // jpd     # DMAs per batch
    # Coarse 2MB chunks keep the PE in long dense bursts (the pstate ramp
    # needs ~3us of continuous execution to reach full clock; fine chunks
    # leave it gap-paced at ~half speed). The last batch tapers to fine
    # chunks so the post-stream tail stays short.
    plan = [[jpd] * nd for _ in range(bl)]
    split_last = taper and jc == 8 and jpd in (4, 8)
    if split_last:
        plan[bl - 1] = [1, 1, 2, 4]

    mm_dt = getattr(mybir.dt, mm_dtype)
    enc_d = nc.declare_dram_parameter("enc", [bl, h, s], mm_dt, isOutput=False)
    u_d = nc.declare_dram_parameter("u", [128, jc], mm_dt, isOutput=False)
    cb_d = nc.declare_dram_parameter("cb", [1, 1], f32, isOutput=False)
    out_d = nc.declare_dram_parameter("out", [bl, s], f32, isOutput=True)

    with tile.TileContext(nc) as tc:
        with (
            tc.tile_pool(name="up", bufs=1) as up,
            tc.tile_pool(name="encp", bufs=enc_bufs) as encp,
            tc.tile_pool(name="smp", bufs=bl) as smp,
            tc.tile_pool(name="psp", bufs=2, space="PSUM") as psp,
        ):
            # Issue the first enc load before anything else so the DMA
            # pipeline starts immediately; the tiny u/cb loads follow it.
            t0 = encp.tile([128, plan[0][0], s], mm_dt, name="t",
                           padded_shape=[128, jpd, s])
            nc.sync.dma_start(
                t0[:],
                enc_d[0, 0:plan[0][0] * 128, :].rearrange("(j p) s -> p j s", p=128),
            )
            u_sb = up.tile([128, jc], mm_dt)
            nc.sync.dma_start(u_sb[:], u_d[:])
            cb_sb = up.tile([1, 1], f32)
            nc.sync.dma_start(cb_sb[:], cb_d[:])

            for b in range(bl):
                # Accumulate this batch's energy row in PSUM [1, s] (4 banks,
                # partition 0); 8 matmuls per 512-wide slice.
                e_ps = psp.tile([1, s], f32)
                p_exp = smp.tile([1, s], f32)
                last = b == bl - 1 and split_last
                j = 0
                for d, cw in enumerate(plan[b]):
                    taper_d = last and d == len(plan[b]) - 1
                    if taper_d:
                        pieces = [(k * MM_N, MM_N) for k in range(ns - 1)]
                        pieces += [((ns - 1) * MM_N, MM_N // 2),
                                   ((ns - 1) * MM_N + MM_N // 2, MM_N // 2)]
                    else:
                        pieces = [(0, s)]
                    for off, w in pieces:
                        if b == 0 and d == 0:
                            t = t0
                            coff = 0
                        else:
                            t = encp.tile([128, cw, w], mm_dt, name="t",
                                          padded_shape=[128, jpd, s])
                            src = enc_d[b, j * 128:(j + cw) * 128,
                                        off:off + w]
                            nc.sync.dma_start(
                                t[:], src.rearrange("(j p) s -> p j s", p=128)
                            )
                            coff = -off
                        for jl in range(cw):
                            if taper_d:
                                subs = [(off, w)]
                            else:
                                subs = [(k * MM_N, MM_N) for k in range(ns)]
                            for soff, sw in subs:
                                nc.tensor.matmul(
                                    e_ps[:, soff:soff + sw],
                                    u_sb[:, j + jl:j + jl + 1],
                                    t[:, jl, soff + coff:soff + coff + sw],
                                    start=(j + jl == 0),
                                    stop=(j + jl == jc - 1),
                                )
                                if j + jl == jc - 1:
                                    # This region's accumulation is complete:
                                    # exp(e - C), overlapping remaining
                                    # matmuls/DMAs.
                                    nc.scalar.activation(
                                        p_exp[:, soff:soff + sw],
                                        e_ps[:, soff:soff + sw],
                                        mybir.ActivationFunctionType.Exp,
                                        bias=cb_sb[:],
                                    )
                                    if last:
                                        nc.scalar.dma_start(
                                            out_d[b:b + 1, soff:soff + sw],
                                            p_exp[:, soff:soff + sw],
                                        )
                    j += cw
                if not last:
                    nc.scalar.dma_start(out_d[b:b + 1, :], p_exp[:])
    if strip_queues:
        # The SWDGE ring (qPoolDynamic) is unused - drop its declaration.
        nc.m.queues = [q for q in nc.m.queues if q.name != "qPoolDynamic"]
    nc.compile()
    return nc


def _prep_inputs(encoder_outputs, attn_W, v, np_dtype=np.float16):
    encoder_outputs = np.asarray(encoder_outputs, dtype=np.float32)
    attn_W = np.asarray(attn_W, dtype=np.float32)
    v = np.asarray(v, dtype=np.float32)
    h = attn_W.shape[0]
    # u = v[0] @ We in float64 (host-side, tiny)
    u64 = v[0].astype(np.float64) @ attn_W[:, h:].astype(np.float64)
    u = u64.astype(np_dtype)
    # energy[b,s] ~ N(0, ||u||^2); C ~ expected row max keeps exp() in range.
    bias_c = 3.6 * float(np.linalg.norm(u64))
    u128 = np.ascontiguousarray(u.reshape(h // 128, 128).T)  # [128, jc]
    cb = np.array([[-bias_c]], dtype=np.float32)
    in_maps = []
    for c in range(NCORES):
        sl = encoder_outputs[:, c * BL:(c + 1) * BL, :]
        enc_c = np.ascontiguousarray(sl.transpose(1, 2, 0).astype(np_dtype))  # [BL,H,S]
        in_maps.append({"enc": enc_c, "u": u128, "cb": cb})
    return in_maps, bias_c


def run(encoder_outputs, rnn_hidden, attn_W, attn_b, v, trace=False,
        mm_dtype="float16", **bass_kwargs):
    np_dtype = {"float16": np.float16, "float32r": np.float32,
                "float32": np.float32}[mm_dtype]
    in_maps, bias_c = _prep_inputs(encoder_outputs, attn_W, v, np_dtype=np_dtype)
    nc = build_nc(mm_dtype=mm_dtype)
    res = run_bass_kernel_spmd(
        nc, in_maps, list(range(NCORES)), trace=trace, **bass_kwargs
    )
    num = np.concatenate([r["out"] for r in res.results], axis=0)  # [B, S]
    tot = num.astype(np.float64).sum(axis=1)                       # [B]
    out = num / tot[:, None]
    return out[:, None, :].astype(np.float32), res


def kernel(encoder_outputs, rnn_hidden, attn_W, attn_b, v):
    out, _ = run(encoder_outputs, rnn_hidden, attn_W, attn_b, v)
    return out
